# revision 11
# baseline (speedup 1.0000x reference)
"""Two-layer GAT on 8 Trainium2 NeuronCores (Bass/Tile SPMD kernel).

Strategy (dst-node graph partitioning):
  - Host partitions the 10000 nodes into 80 balanced blocks (<=128 nodes each,
    ~equal in-edge counts); core c owns blocks [10c, 10c+10) = 1280 node slots.
    Blocks 0-4 of each core form half A (slots 0:640), blocks 5-9 half B.
  - Per layer: each core computes h = x_shard @ [W | W@a_src | W@a_dst] on the
    TensorEngine (fp16), builds H_aug = [h | 1.0 | e_src(f32)] rows, and
    AllGathers the two half-slabs as two collectives (A then B) so the edge
    phase for A-sourced edges overlaps the B AllGather.
  - Edge phase per 128-dst block: dma_gather of the source rows of all in-edges
    (edges sorted by dst and split by source half; indices are host-precomputed
    int16), per-edge attention logits via one-hot matmuls (S^T @ e_dst
    broadcast), then the weighted segment-sum as a matmul with the w-scaled
    one-hot matrix:
        out[d,:] = sum_e w_e * S[e,d] * [h|1][src_e]  (PSUM accumulate)
    The trailing ones-column yields z[d] = sum_e w_e, and the softmax
    normalization happens after aggregation: out = unnorm * (1/z).
  - exp() needs no max-subtraction: logits are bounded (|e| < ~10) in f32.
"""

import heapq
import math
import numpy as np

import concourse.bass as bass
import concourse.bacc as bacc
import concourse.mybir as mybir
import concourse.tile as tile
from concourse import bass_utils

F16 = mybir.dt.float16
F32 = mybir.dt.float32
I16 = mybir.dt.int16

P = 128  # partitions


class GATConfig:
    def __init__(self, n_nodes, n_edges, d, ncores=8, blk_per_core=10):
        self.N = n_nodes
        self.E = n_edges
        self.D = d
        self.NCORES = ncores
        self.BPC = blk_per_core
        assert blk_per_core % 2 == 0
        self.HBPC = blk_per_core // 2
        self.NBLK = ncores * blk_per_core
        self.SLOTS = blk_per_core * P  # node slots per core
        self.HALF = self.SLOTS // 2
        self.KT = d // P  # contraction tiles
        self.HCOLS = d + P  # H_aug row: [h(D) | 1.0 | pad | e_src f32 | 0...]
        assert (self.HCOLS * 2) % 256 == 0
        assert d % P == 0
        self.CA = None  # per-block-index chunk counts for src-half A
        self.CB = None


def _partition_graph(cfg, src, dst):
    """Assign nodes to NBLK balanced blocks (<=128 nodes, ~equal edges)."""
    N, NBLK = cfg.N, cfg.NBLK
    deg = np.bincount(dst, minlength=N)
    order = np.argsort(-deg, kind="stable")
    blk_edges = np.zeros(NBLK, dtype=np.int64)
    blk_count = np.zeros(NBLK, dtype=np.int64)
    node_blk = np.empty(N, dtype=np.int64)
    node_slot = np.empty(N, dtype=np.int64)
    heap = [(0, b) for b in range(NBLK)]
    heapq.heapify(heap)
    for n in order:
        while True:
            e, b = heapq.heappop(heap)
            if e == blk_edges[b] and blk_count[b] < P:
                break
        node_blk[n] = b
        node_slot[n] = blk_count[b]
        blk_count[b] += 1
        blk_edges[b] += deg[n]
        heapq.heappush(heap, (int(blk_edges[b]), b))

    node_core = node_blk // cfg.BPC
    node_loc = node_core * cfg.SLOTS + (node_blk % cfg.BPC) * P + node_slot
    # gather-array row: per half (A = blocks 0..HBPC-1 of each core)
    lb = node_blk % cfg.BPC
    node_half = (lb >= cfg.HBPC).astype(np.int64)
    node_hloc = node_core * cfg.HALF + (lb - node_half * cfg.HBPC) * P + node_slot
    return node_loc, node_blk, node_slot, blk_count, node_half, node_hloc


def _build_edge_data(cfg, src, dst, parts):
    """Per-core gather indices + one-hot S / S^T matrices (fp16),
    with each block's edges split by source half (A chunks then B chunks)."""
    node_loc, node_blk, node_slot, blk_count, node_half, node_hloc = parts
    BPC, HBPC = cfg.BPC, cfg.HBPC

    # first pass: per-(core, block, group) real edge counts -> per-block C
    e_core = node_blk[dst] // BPC
    e_blk = node_blk[dst] % BPC
    e_half = node_half[src]
    cntA = np.zeros((cfg.NCORES, BPC), dtype=np.int64)
    cntB = np.zeros((cfg.NCORES, BPC), dtype=np.int64)
    np.add.at(cntA, (e_core[e_half == 0], e_blk[e_half == 0]), 1)
    np.add.at(cntB, (e_core[e_half == 1], e_blk[e_half == 1]), 1)
    # pad-slot dummies go to the smaller group per (core, block); for the
    # uniform per-block-index capacity just add them to both maxima.
    npad = (P - blk_count.reshape(cfg.NCORES, BPC)).astype(np.int64)
    CA = np.maximum(np.ceil((cntA + npad) / P), 1).astype(np.int64).max(axis=0)
    CB = np.maximum(np.ceil((cntB + npad) / P), 1).astype(np.int64).max(axis=0)
    cfg.CA = [int(x) for x in CA]
    cfg.CB = [int(x) for x in CB]
    CTOT = int(CA.sum() + CB.sum())  # chunks per core per layer
    cfg.CTOT = CTOT

    cores = []
    for c in range(cfg.NCORES):
        mask = e_core == c
        es, eb, eslot, eh = (
            src[mask],
            e_blk[mask],
            node_slot[dst[mask]],
            e_half[mask],
        )
        src_loc = node_hloc[es]

        idx_tile = np.zeros((16, CTOT * 8), dtype=np.int16)
        S = np.zeros((P, CTOT * P), dtype=np.float16)
        ST = np.zeros((P, CTOT * P), dtype=np.float16)
        ch0 = 0
        for b in range(BPC):
            gb = c * BPC + b
            pad_slots = list(range(int(blk_count[gb]), P))
            for g, Cg in ((0, cfg.CA[b]), (1, cfg.CB[b])):
                bm = (eb == b) & (eh == g)
                ne = int(bm.sum())
                cap = Cg * P
                locs = np.zeros(cap, dtype=np.int16)
                dsl = np.full(cap, -1, dtype=np.int64)
                locs[:ne] = src_loc[bm]
                dsl[:ne] = eslot[bm]
                # dummy edges so empty slots get z > 0 (avoid 1/0): group A
                # takes them (A always has >= npad slack by construction)
                if g == 0 and pad_slots:
                    k = len(pad_slots)
                    assert ne + k <= cap, (c, b, ne, k, cap)
                    dsl[ne : ne + k] = np.asarray(pad_slots)
                pos = np.arange(cap)
                # idx tile wrap: position i -> [i%16, i//16]
                idx_tile[:, ch0 * 8 : ch0 * 8 + Cg * 8] = locs.reshape(
                    Cg * 8, 16
                ).T
                val = dsl >= 0
                col = (ch0 + pos // P) * P
                S[pos[val] % P, col[val] + dsl[val]] = 1.0
                ST[dsl[val], col[val] + pos[val] % P] = 1.0
                ch0 += Cg
        assert ch0 == CTOT
        cores.append((np.tile(idx_tile, (8, 1)), S, ST))
    return cores


def _layout_xT(cfg, x_perm):
    """[SLOTS, D] f32 -> fp16 tile [128, KT*SLOTS]: [p, k*SLOTS+n] = x[n, k*128+p]."""
    xT = np.ascontiguousarray(x_perm.T).astype(np.float16)  # [D, SLOTS]
    return xT.reshape(cfg.KT, P, cfg.SLOTS).transpose(1, 0, 2).reshape(P, -1)


def _layout_W(cfg, W, a_src, a_dst):
    """[W | W@a_src | W@a_dst] -> fp16 tile [128, KT*(D+2)]."""
    Wx = np.concatenate(
        [W, (W @ a_src)[:, None], (W @ a_dst)[:, None]], axis=1
    ).astype(np.float16)
    return Wx.reshape(cfg.KT, P, cfg.D + 2).transpose(1, 0, 2).reshape(P, -1)


def _build_program(cfg, with_bias):
    """Build the SPMD Bass program (identical on all 8 cores)."""
    nc = bacc.Bacc(
        "TRN2", target_bir_lowering=False, debug=False, num_devices=cfg.NCORES
    )
    D, KT, BPC, SLOTS, HCOLS = cfg.D, cfg.KT, cfg.BPC, cfg.SLOTS, cfg.HCOLS
    HBPC, HALF = cfg.HBPC, cfg.HALF
    CA, CB, CTOT = cfg.CA, cfg.CB, cfg.CTOT
    DC = D + 2  # dense output cols: h | e_src | e_dst
    ACOL = D + 1  # agg matmul cols: h | ones

    xT_in = nc.dram_tensor("xT", [P, KT * SLOTS], F16, kind="ExternalInput")
    w_in = [
        nc.dram_tensor(f"w{L}", [P, KT * DC], F16, kind="ExternalInput")
        for L in range(2)
    ]
    s_in = nc.dram_tensor("s", [P, CTOT * P], F16, kind="ExternalInput")
    st_in = nc.dram_tensor("st", [P, CTOT * P], F16, kind="ExternalInput")
    idx_in = nc.dram_tensor("idx", [P, CTOT * 8], I16, kind="ExternalInput")
    if with_bias:
        b_in = [
            nc.dram_tensor(f"b{L}", [P, D], F32, kind="ExternalInput")
            for L in range(2)
        ]
    out_dram = nc.dram_tensor("out", [SLOTS, D], F32, kind="ExternalOutput")

    def nsplits(total):
        spl, o = [], 0
        while o < total:
            w = min(512, total - o)
            spl.append((o, w))
            o += w
        return spl

    # block b's first chunk index and per-group chunk counts
    blk_ch0 = []
    ch = 0
    for b in range(BPC):
        blk_ch0.append(ch)
        ch += CA[b] + CB[b]

    with tile.TileContext(nc) as tc:
        with (
            tc.tile_pool(name="const", bufs=1) as cpool,
            tc.tile_pool(name="work", bufs=3) as wpool,
            tc.tile_pool(name="gather", bufs=2) as gpool,
            tc.tile_pool(name="psum", bufs=3, space="PSUM") as ppool,
            tc.tile_pool(name="dram", bufs=1, space="DRAM") as dpool,
        ):
            xT1_sb = cpool.tile([P, KT * SLOTS], F16)
            x2T_sb = cpool.tile([P, KT * SLOTS], F16)
            w_sb = [cpool.tile([P, KT * DC], F16, name=f"w_sb{L}") for L in range(2)]
            idx_sb = cpool.tile([P, CTOT * 8], I16)
            edst_sb = [
                cpool.tile([P, BPC], F16, name=f"edst_sb{L}") for L in range(2)
            ]
            if with_bias:
                b_sb = [cpool.tile([P, D], F32, name=f"b_sb{L}") for L in range(2)]

            nc.sync.dma_start(xT1_sb[:], xT_in[:])
            for L in range(2):
                nc.sync.dma_start(w_sb[L][:], w_in[L][:])
                if with_bias:
                    nc.sync.dma_start(b_sb[L][:], b_in[L][:])
            nc.sync.dma_start(idx_sb[:], idx_in[:])
            maxC = max(CA[x] + CB[x] for x in range(BPC))

            ag_in = [
                [
                    dpool.tile([HALF, HCOLS], F16, name=f"ag_in{L}{g}")
                    for g in range(2)
                ]
                for L in range(2)
            ]
            ag_out = [
                [
                    dpool.tile(
                        [cfg.NCORES * HALF, HCOLS],
                        F16,
                        name=f"ag_out{L}{g}",
                        addr_space="Shared",
                    )
                    for g in range(2)
                ]
                for L in range(2)
            ]

            for L in range(2):
                xT = xT1_sb if L == 0 else x2T_sb
                # ---- dense phase: psum[:, 0:D]=h, D=e_src, D+1=e_dst ----
                for m in range(BPC):
                    g, mh = (0, m) if m < HBPC else (1, m - HBPC)
                    ps = ppool.tile([P, DC], F32, name="ps_main", padded_shape=[P, 772])
                    for k in range(KT):
                        lhs = xT[:, k * SLOTS + m * P : k * SLOTS + (m + 1) * P]
                        for o, w in nsplits(DC):
                            nc.tensor.matmul(
                                ps[:, o : o + w],
                                lhs,
                                w_sb[L][:, k * DC + o : k * DC + o + w],
                                start=(k == 0),
                                stop=(k == KT - 1),
                            )
                    hst = wpool.tile([P, HCOLS], F16, name="hst")
                    if with_bias and L == 1:
                        tmp = wpool.tile([P, D], F32, name="hb_tmp")
                        nc.vector.tensor_add(tmp[:], ps[:, 0:D], b_sb[1][:])
                        nc.vector.tensor_copy(hst[:, 0:D], tmp[:])
                    else:
                        nc.vector.tensor_copy(hst[:, 0:D], ps[:, 0:D])
                    nc.vector.memset(hst[:, D : D + 1], 1.0)  # ones col for z
                    nc.vector.memset(hst[:, D + 1 : D + 2], 0.0)
                    nc.vector.memset(hst[:, D + 4 : HCOLS], 0.0)
                    hf32 = hst.bitcast(F32)  # [P, HCOLS//2]
                    nc.vector.tensor_copy(
                        hf32[:, D // 2 + 1 : D // 2 + 2], ps[:, D : D + 1]
                    )  # e_src exact f32
                    nc.vector.tensor_copy(
                        edst_sb[L][:, m : m + 1], ps[:, D + 1 : D + 2]
                    )
                    nc.sync.dma_start(
                        ag_in[L][g][mh * P : (mh + 1) * P, :], hst[:]
                    )

                for g in range(2):
                    nc.gpsimd.collective_compute(
                        "AllGather",
                        mybir.AluOpType.bypass,
                        replica_groups=[list(range(cfg.NCORES))],
                        ins=[ag_in[L][g].opt()],
                        outs=[ag_out[L][g].opt()],
                    )

                # ---- edge phase ----
                for b in range(BPC):
                    C = CA[b] + CB[b]
                    ch0 = blk_ch0[b]
                    gt = gpool.tile(
                        [P, C, HCOLS], F16, name="gt",
                        padded_shape=[P, maxC, HCOLS],
                    )
                    s_blk = wpool.tile(
                        [P, C * P], F16, name="s_blk", bufs=2,
                        padded_shape=[P, maxC * P],
                    )
                    nc.sync.dma_start(
                        s_blk[:], s_in[:, ch0 * P : (ch0 + C) * P]
                    )
                    st_blk = wpool.tile(
                        [P, C * P], F16, name="st_blk", bufs=2,
                        padded_shape=[P, maxC * P],
                    )
                    nc.sync.dma_start(
                        st_blk[:], st_in[:, ch0 * P : (ch0 + C) * P]
                    )
                    for g, g0, gC in ((0, 0, CA[b]), (1, CA[b], CB[b])):
                        nc.gpsimd.dma_gather(
                            gt[:, g0 : g0 + gC, :],
                            ag_out[L][g][:],
                            idx_sb[:, (ch0 + g0) * 8 : (ch0 + g0 + gC) * 8],
                            num_idxs=gC * P,
                            num_idxs_reg=gC * P,
                            elem_size=HCOLS,
                            single_packet=False,
                        )
                    # e_dst per edge: S^T[d,e] @ e_dst_block
                    pe = ppool.tile([P, C], F32, name="ps_edst", bufs=2,
                                    padded_shape=[P, 16])
                    for c in range(C):
                        nc.tensor.matmul(
                            pe[:, c : c + 1],
                            st_blk[:, c * P : (c + 1) * P],
                            edst_sb[L][:, b : b + 1],
                            start=True,
                            stop=True,
                        )
                    # w = exp(leakyrelu(e_src + e_dst))
                    gt32 = gt.bitcast(F32)  # [P, C, HCOLS//2]
                    esrc = gt32[:, :, D // 2 + 1 : D // 2 + 2].rearrange(
                        "p c one -> p (c one)"
                    )
                    e0 = wpool.tile([P, C], F32, name="e0", padded_shape=[P, 16])
                    nc.vector.tensor_add(e0[:], esrc, pe[:])
                    e2 = wpool.tile([P, C], F32, name="e2", padded_shape=[P, 16])
                    nc.vector.scalar_tensor_tensor(
                        e2[:], e0[:], 0.2, e0[:],
                        op0=mybir.AluOpType.mult, op1=mybir.AluOpType.max,
                    )
                    wh = wpool.tile([P, C], F32, name="wh", padded_shape=[P, 16])
                    nc.scalar.activation(
                        wh[:], e2[:], mybir.ActivationFunctionType.Exp
                    )
                    # aggregation: (w*S)^T @ [h|1] accumulated over chunks
                    psb = ppool.tile(
                        [P, ACOL], F32, name="ps_main", padded_shape=[P, 772]
                    )
                    for c in range(C):
                        sw = wpool.tile([P, P], F16, name="sw", bufs=4)
                        nc.scalar.activation(
                            sw[:],
                            s_blk[:, c * P : (c + 1) * P],
                            mybir.ActivationFunctionType.Copy,
                            scale=wh[:, c : c + 1],
                        )
                        for o, w in nsplits(ACOL):
                            nc.tensor.matmul(
                                psb[:, o : o + w],
                                sw[:],
                                gt[:, c, o : o + w],
                                start=(c == 0),
                                stop=(c == C - 1),
                            )
                    rz = wpool.tile([P, 1], F32, name="rz")
                    nc.vector.reciprocal(rz[:], psb[:, D : D + 1])
                    if L == 0:
                        x2 = wpool.tile([P, D], F16, name="x2")
                        if with_bias:
                            t0 = wpool.tile([P, D], F32, name="bt0")
                            nc.vector.tensor_scalar(
                                t0[:], psb[:, 0:D], rz[:], None,
                                mybir.AluOpType.mult,
                            )
                            t1 = wpool.tile([P, D], F32, name="bt1")
                            nc.vector.tensor_add(t1[:], t0[:], b_sb[0][:])
                            nc.vector.tensor_scalar(
                                x2[:], t1[:], 0.0, None, mybir.AluOpType.max
                            )
                        else:
                            nc.scalar.activation(
                                x2[:],
                                psb[:, 0:D],
                                mybir.ActivationFunctionType.Relu,
                                scale=rz[:],
                            )
                        for kt in range(KT):
                            nc.sync.dma_start(
                                x2T_sb[
                                    :, kt * SLOTS + b * P : kt * SLOTS + (b + 1) * P
                                ],
                                x2[:, kt * P : (kt + 1) * P],
                                transpose=True,
                            )
                    else:
                        of = wpool.tile([P, D], F32, name="of")
                        nc.scalar.activation(
                            of[:],
                            psb[:, 0:D],
                            mybir.ActivationFunctionType.Identity,
                            scale=rz[:],
                        )
                        nc.sync.dma_start(out_dram[b * P : (b + 1) * P, :], of[:])

    nc.compile()
    return nc


def _host_prep(cfg, x, edge_index, params):
    """Build per-core in_maps + unpermute info."""
    N = cfg.N
    src = np.concatenate([edge_index[0], np.arange(N, dtype=edge_index.dtype)])
    dst = np.concatenate([edge_index[1], np.arange(N, dtype=edge_index.dtype)])
    src = src.astype(np.int64)
    dst = dst.astype(np.int64)
    parts = _partition_graph(cfg, src, dst)
    node_loc = parts[0]
    edge_data = _build_edge_data(cfg, src, dst, parts)

    x_perm_all = np.zeros((cfg.NCORES * cfg.SLOTS, cfg.D), dtype=np.float32)
    x_perm_all[node_loc] = x

    W1, a_s1, a_d1, b1, W2, a_s2, a_d2, b2 = params
    w_tiles = [_layout_W(cfg, W1, a_s1, a_d1), _layout_W(cfg, W2, a_s2, a_d2)]
    with_bias = bool(np.any(b1) or np.any(b2))

    in_maps = []
    for c in range(cfg.NCORES):
        idx_tile, S, ST = edge_data[c]
        m = {
            "xT": _layout_xT(cfg, x_perm_all[c * cfg.SLOTS : (c + 1) * cfg.SLOTS]),
            "w0": w_tiles[0],
            "w1": w_tiles[1],
            "s": S,
            "st": ST,
            "idx": idx_tile,
        }
        if with_bias:
            m["b0"] = np.tile(b1.astype(np.float32)[None, :], (P, 1))
            m["b1"] = np.tile(b2.astype(np.float32)[None, :], (P, 1))
        in_maps.append(m)
    return in_maps, node_loc, with_bias


_PROGRAM_CACHE = {}


def run_gat(cfg, x, edge_index, params, trace=False):
    in_maps, node_loc, with_bias = _host_prep(cfg, x, edge_index, params)
    key = (cfg.N, cfg.E, cfg.D, tuple(cfg.CA), tuple(cfg.CB), with_bias)
    if key not in _PROGRAM_CACHE:
        _PROGRAM_CACHE[key] = _build_program(cfg, with_bias)
    nc = _PROGRAM_CACHE[key]
    res = bass_utils.run_bass_kernel_spmd(
        nc, in_maps, core_ids=list(range(cfg.NCORES)), trace=trace
    )
    outs = np.concatenate([res.results[c]["out"] for c in range(cfg.NCORES)], axis=0)
    full = outs[node_loc]  # unpermute (drops pad slots)
    return full.astype(np.float32), res


def kernel(x, edge_index, W1, att_src1, att_dst1, b1, W2, att_src2, att_dst2, b2):
    x = np.asarray(x, dtype=np.float32)
    edge_index = np.asarray(edge_index)
    cfg = GATConfig(x.shape[0], edge_index.shape[1], x.shape[1])
    params = tuple(
        np.asarray(p, dtype=np.float32)
        for p in (W1, att_src1, att_dst1, b1, W2, att_src2, att_dst2, b2)
    )
    out, _ = run_gat(cfg, x, edge_index, params)
    return out


# revision 12
# speedup vs baseline: 1.2814x; 1.2814x over previous
"""Two-layer GAT on 8 Trainium2 NeuronCores (Bass/Tile SPMD kernel).

Strategy (dst-node graph partitioning):
  - Host partitions the 10000 nodes into 80 balanced blocks (<=128 nodes each,
    ~equal in-edge counts); core c owns blocks [10c, 10c+10) = 1280 node slots.
  - Per layer: each core computes h = x_shard @ [W | W@a_src | W@a_dst] on the
    TensorEngine (fp16), builds H_aug = [h | 1.0 | e_src(f32)] rows, AllGathers
    H_aug so every core holds all rows in HBM.
  - Edge phase per 128-dst block: dma_gather of the source rows of all in-edges
    (edges sorted by dst; indices are host-precomputed int16), per-edge
    attention logits via one-hot matmuls (S^T @ e_dst broadcast), then the
    weighted segment-sum as a matmul with the w-scaled one-hot matrix:
        out[d,:] = sum_e w_e * S[e,d] * [h|1][src_e]  (PSUM accumulate)
    The trailing ones-column yields z[d] = sum_e w_e, and the softmax
    normalization happens after aggregation: out = unnorm * (1/z).
  - exp() needs no max-subtraction: logits are bounded (|e| < ~10) in f32.
"""

import heapq
import math
import numpy as np

import concourse.bass as bass
import concourse.bacc as bacc
import concourse.mybir as mybir
import concourse.tile as tile
from concourse import bass_utils

F16 = mybir.dt.float16
F32 = mybir.dt.float32
I16 = mybir.dt.int16

P = 128  # partitions


class GATConfig:
    def __init__(self, n_nodes, n_edges, d, ncores=8, blk_per_core=10):
        self.N = n_nodes
        self.E = n_edges
        self.D = d
        self.NCORES = ncores
        self.BPC = blk_per_core
        self.NBLK = ncores * blk_per_core
        self.SLOTS = blk_per_core * P  # node slots per core
        self.KT = d // P  # contraction tiles
        self.HCOLS = d + P  # H_aug row: [h(D) | 1.0 | pad | e_src f32 | 0...]
        assert (self.HCOLS * 2) % 256 == 0
        assert d % P == 0
        self.C = None  # chunks per block (set after partitioning)


# Default (real problem) config
CFG = GATConfig(10000, 100000, 768)


def _partition_graph(cfg, src, dst):
    """Assign nodes to NBLK balanced blocks (<=128 nodes, ~equal edges).

    Returns node_loc (H_aug row of each node), per-(core,block) edge arrays.
    """
    N, NBLK = cfg.N, cfg.NBLK
    deg = np.bincount(dst, minlength=N)
    order = np.argsort(-deg, kind="stable")
    blk_edges = np.zeros(NBLK, dtype=np.int64)
    blk_count = np.zeros(NBLK, dtype=np.int64)
    node_blk = np.empty(N, dtype=np.int64)
    node_slot = np.empty(N, dtype=np.int64)
    heap = [(0, b) for b in range(NBLK)]
    heapq.heapify(heap)
    for n in order:
        while True:
            e, b = heapq.heappop(heap)
            if e == blk_edges[b] and blk_count[b] < P:
                break
        node_blk[n] = b
        node_slot[n] = blk_count[b]
        blk_count[b] += 1
        blk_edges[b] += deg[n]
        heapq.heappush(heap, (int(blk_edges[b]), b))

    # capacity must cover real edges + one dummy edge per empty slot
    need = blk_edges + (P - blk_count)
    C = int(math.ceil(need.max() / P))
    cfg.C = C

    node_core = node_blk // cfg.BPC
    node_loc = node_core * cfg.SLOTS + (node_blk % cfg.BPC) * P + node_slot
    return node_loc, node_blk, node_slot, blk_count, C


def _build_edge_data(cfg, src, dst, node_loc, node_blk, node_slot, blk_count):
    """Per-core gather indices + one-hot S / S^T matrices (fp16)."""
    C = cfg.C
    CP = C * P
    cores = []
    e_blk_all = node_blk[dst]
    for c in range(cfg.NCORES):
        mask = (e_blk_all // cfg.BPC) == c
        es, ed = src[mask], dst[mask]
        eb = node_blk[ed] % cfg.BPC
        eslot = node_slot[ed]
        ord_ = np.lexsort((eslot, eb))
        es, eb, eslot = es[ord_], eb[ord_], eslot[ord_]
        src_loc = node_loc[es]

        idx_arr = np.zeros((cfg.BPC, CP), dtype=np.int16)
        dslot_arr = np.full((cfg.BPC, CP), -1, dtype=np.int64)
        for b in range(cfg.BPC):
            bm = eb == b
            ne = int(bm.sum())
            gb = c * cfg.BPC + b
            npad_slots = P - int(blk_count[gb])
            assert ne + npad_slots <= CP, (ne, npad_slots, CP)
            idx_arr[b, :ne] = src_loc[bm]
            dslot_arr[b, :ne] = eslot[bm]
            # dummy edges so empty slots get z > 0 (avoid 1/0)
            if npad_slots:
                dslot_arr[b, ne : ne + npad_slots] = np.arange(
                    blk_count[gb], P, dtype=np.int64
                )
            # remaining filler edges keep dslot=-1 -> all-zero S row

        # gather index tile: position i of block b -> [16r + i%16, b*CP/16 + i/16]
        idx_tile = (
            idx_arr.reshape(cfg.BPC, CP // 16, 16)
            .transpose(2, 0, 1)
            .reshape(16, cfg.BPC * (CP // 16))
        )
        idx_tile = np.tile(idx_tile, (8, 1))  # replicate to 128 partitions

        # S: [128(edge-in-chunk), NCHUNK*128(dst-slot)], S^T mirrored
        NCH = cfg.BPC * C
        S = np.zeros((P, NCH * P), dtype=np.float16)
        ST = np.zeros((P, NCH * P), dtype=np.float16)
        b_i, pos_i = np.nonzero(dslot_arr >= 0)
        dsl = dslot_arr[b_i, pos_i]
        ch = b_i * C + pos_i // P
        pp = pos_i % P
        S[pp, ch * P + dsl] = 1.0
        ST[dsl, ch * P + pp] = 1.0
        cores.append((idx_tile, S, ST))
    return cores


def _layout_xT(cfg, x_perm):
    """[SLOTS, D] f32 -> fp16 tile [128, KT*SLOTS]: [p, k*SLOTS+n] = x[n, k*128+p]."""
    xT = np.ascontiguousarray(x_perm.T).astype(np.float16)  # [D, SLOTS]
    return xT.reshape(cfg.KT, P, cfg.SLOTS).transpose(1, 0, 2).reshape(P, -1)


def _layout_W(cfg, W, a_src, a_dst):
    """[W | W@a_src | W@a_dst] -> fp16 tile [128, KT*(D+2)]."""
    Wx = np.concatenate(
        [
            W,
            (W @ a_src)[:, None],
            (W @ a_dst)[:, None],
        ],
        axis=1,
    ).astype(np.float16)  # [D, D+2]
    return Wx.reshape(cfg.KT, P, cfg.D + 2).transpose(1, 0, 2).reshape(P, -1)


def _build_program(cfg, with_bias):
    """Build the SPMD Bass program (identical on all 8 cores)."""
    nc = bacc.Bacc(
        "TRN2", target_bir_lowering=False, debug=False, num_devices=cfg.NCORES
    )
    D, KT, C, BPC, SLOTS, HCOLS = cfg.D, cfg.KT, cfg.C, cfg.BPC, cfg.SLOTS, cfg.HCOLS
    NCH = BPC * C
    DC = D + 2  # dense output cols: h | e_src | e_dst
    ACOL = D + 1  # agg matmul cols: h | ones

    xT_in = nc.dram_tensor("xT", [P, KT * SLOTS], F16, kind="ExternalInput")
    w_in = [
        nc.dram_tensor(f"w{L}", [P, KT * DC], F16, kind="ExternalInput")
        for L in range(2)
    ]
    s_in = nc.dram_tensor("s", [P, NCH * P], F16, kind="ExternalInput")
    st_in = nc.dram_tensor("st", [P, NCH * P], F16, kind="ExternalInput")
    idx_in = nc.dram_tensor("idx", [P, NCH * 8], I16, kind="ExternalInput")
    ident_in = nc.dram_tensor("ident", [P, P], F16, kind="ExternalInput")
    if with_bias:
        b_in = [
            nc.dram_tensor(f"b{L}", [P, D], F32, kind="ExternalInput")
            for L in range(2)
        ]
    out_dram = nc.dram_tensor("out", [SLOTS, D], F32, kind="ExternalOutput")

    # column split for <=512-wide matmuls
    def nsplits(total):
        spl = []
        o = 0
        while o < total:
            w = min(512, total - o)
            spl.append((o, w))
            o += w
        return spl

    with tile.TileContext(nc) as tc:
        with (
            tc.tile_pool(name="const", bufs=1) as cpool,
            tc.tile_pool(name="work", bufs=3) as wpool,
            tc.tile_pool(name="gather", bufs=2) as gpool,
            tc.tile_pool(name="psum", bufs=2, space="PSUM") as ppool,
            tc.tile_pool(name="dram", bufs=1, space="DRAM") as dpool,
        ):
            xT1_sb = cpool.tile([P, KT * SLOTS], F16)
            x2T_sb = cpool.tile([P, KT * SLOTS], F16)
            w_sb = [cpool.tile([P, KT * DC], F16, name=f"w_sb{L}") for L in range(2)]
            s_sb = cpool.tile([P, NCH * P], F16)
            st_sb = cpool.tile([P, NCH * P], F16)
            idx_sb = cpool.tile([P, NCH * 8], I16)
            ident_sb = cpool.tile([P, P], F16)
            edst_sb = [
                cpool.tile([P, BPC], F16, name=f"edst_sb{L}") for L in range(2)
            ]
            if with_bias:
                b_sb = [cpool.tile([P, D], F32, name=f"b_sb{L}") for L in range(2)]

            nc.sync.dma_start(xT1_sb[:], xT_in[:])
            for L in range(2):
                nc.sync.dma_start(w_sb[L][:], w_in[L][:])
                if with_bias:
                    nc.sync.dma_start(b_sb[L][:], b_in[L][:])
            nc.sync.dma_start(s_sb[:], s_in[:])
            nc.sync.dma_start(st_sb[:], st_in[:])
            nc.sync.dma_start(idx_sb[:], idx_in[:])
            nc.sync.dma_start(ident_sb[:], ident_in[:])

            ag_in = [
                dpool.tile([SLOTS, HCOLS], F16, name=f"ag_in{L}") for L in range(2)
            ]
            ag_out = [
                dpool.tile(
                    [cfg.NCORES * SLOTS, HCOLS],
                    F16,
                    name=f"ag_out{L}",
                    addr_space="Shared",
                )
                for L in range(2)
            ]

            for L in range(2):
                xT = xT1_sb if L == 0 else x2T_sb
                # ---- dense phase: psum[:, 0:D]=h, D=e_src, D+1=e_dst ----
                for m in range(BPC):
                    ps = ppool.tile([P, DC], F32, name="ps_main", padded_shape=[P, 772])
                    for k in range(KT):
                        lhs = xT[:, k * SLOTS + m * P : k * SLOTS + (m + 1) * P]
                        for o, w in nsplits(DC):
                            nc.tensor.matmul(
                                ps[:, o : o + w],
                                lhs,
                                w_sb[L][:, k * DC + o : k * DC + o + w],
                                start=(k == 0),
                                stop=(k == KT - 1),
                            )
                    hst = wpool.tile([P, HCOLS], F16, name="hst")
                    if with_bias and L == 1:
                        # layer-2 bias folds into stored h (sum(alpha)=1)
                        tmp = wpool.tile([P, D], F32, name="hb_tmp")
                        nc.vector.tensor_add(tmp[:], ps[:, 0:D], b_sb[1][:])
                        nc.vector.tensor_copy(hst[:, 0:D], tmp[:])
                    else:
                        nc.vector.tensor_copy(hst[:, 0:D], ps[:, 0:D])
                    nc.vector.memset(hst[:, D : D + 1], 1.0)  # ones col for z
                    nc.vector.memset(hst[:, D + 1 : D + 2], 0.0)
                    nc.vector.memset(hst[:, D + 4 : HCOLS], 0.0)
                    hf32 = hst.bitcast(F32)  # [P, HCOLS//2]
                    nc.vector.tensor_copy(
                        hf32[:, D // 2 + 1 : D // 2 + 2], ps[:, D : D + 1]
                    )  # e_src exact f32
                    nc.vector.tensor_copy(
                        edst_sb[L][:, m : m + 1], ps[:, D + 1 : D + 2]
                    )
                    nc.sync.dma_start(ag_in[L][m * P : (m + 1) * P, :], hst[:])

                nc.gpsimd.collective_compute(
                    "AllGather",
                    mybir.AluOpType.bypass,
                    replica_groups=[list(range(cfg.NCORES))],
                    ins=[ag_in[L].opt()],
                    outs=[ag_out[L].opt()],
                )

                # ---- edge phase ----
                GSPLIT = 6  # <=768 idxs per dma_gather (SWDGE ring holds ~1024)
                for b in range(BPC):
                    gt = gpool.tile([P, C, HCOLS], F16, name="gt")
                    for g0 in range(0, C, GSPLIT):
                        g1 = min(g0 + GSPLIT, C)
                        nc.gpsimd.dma_gather(
                            gt[:, g0:g1, :],
                            ag_out[L][:],
                            idx_sb[:, (b * C + g0) * 8 : (b * C + g1) * 8],
                            num_idxs=(g1 - g0) * P,
                            num_idxs_reg=(g1 - g0) * P,
                            elem_size=HCOLS,
                            single_packet=False,
                        )
                    # e_dst per edge: S^T[d,e] @ e_dst_block
                    pe = ppool.tile([P, C], F32, name="ps_edst")
                    for c in range(C):
                        nc.tensor.matmul(
                            pe[:, c : c + 1],
                            st_sb[:, (b * C + c) * P : (b * C + c + 1) * P],
                            edst_sb[L][:, b : b + 1],
                            start=True,
                            stop=True,
                        )
                    # w = exp(leakyrelu(e_src + e_dst))
                    gt32 = gt.bitcast(F32)  # [P, C, HCOLS//2]
                    esrc = gt32[:, :, D // 2 + 1 : D // 2 + 2].rearrange(
                        "p c one -> p (c one)"
                    )
                    e0 = wpool.tile([P, C], F32, name="e0")
                    nc.vector.tensor_add(e0[:], esrc, pe[:])
                    e2 = wpool.tile([P, C], F32, name="e2")
                    # leakyrelu fused: (e0*0.2) max e0
                    nc.vector.scalar_tensor_tensor(
                        e2[:],
                        e0[:],
                        0.2,
                        e0[:],
                        op0=mybir.AluOpType.mult,
                        op1=mybir.AluOpType.max,
                    )
                    wh = wpool.tile([P, C], F32, name="wh")
                    nc.scalar.activation(
                        wh[:], e2[:], mybir.ActivationFunctionType.Exp
                    )
                    # aggregation: (w*S)^T @ [h|1] accumulated over chunks
                    psb = ppool.tile(
                        [P, ACOL], F32, name="ps_main", padded_shape=[P, 772]
                    )
                    for c in range(C):
                        sw = wpool.tile([P, P], F16, name="sw", bufs=4)
                        # per-partition w scale on the (idle) scalar engine
                        nc.scalar.activation(
                            sw[:],
                            s_sb[:, (b * C + c) * P : (b * C + c + 1) * P],
                            mybir.ActivationFunctionType.Copy,
                            scale=wh[:, c : c + 1],
                        )
                        for o, w in nsplits(ACOL):
                            nc.tensor.matmul(
                                psb[:, o : o + w],
                                sw[:],
                                gt[:, c, o : o + w],
                                start=(c == 0),
                                stop=(c == C - 1),
                            )
                    rz = wpool.tile([P, 1], F32, name="rz")
                    nc.vector.reciprocal(rz[:], psb[:, D : D + 1])
                    if L == 0:
                        x2 = wpool.tile([P, D], F16, name="x2")
                        if with_bias:
                            t0 = wpool.tile([P, D], F32, name="bt0")
                            nc.vector.tensor_scalar(
                                t0[:], psb[:, 0:D], rz[:], None, mybir.AluOpType.mult
                            )
                            t1 = wpool.tile([P, D], F32, name="bt1")
                            nc.vector.tensor_add(t1[:], t0[:], b_sb[0][:])
                            nc.vector.tensor_scalar(
                                x2[:], t1[:], 0.0, None, mybir.AluOpType.max
                            )
                        else:
                            nc.scalar.activation(
                                x2[:],
                                psb[:, 0:D],
                                mybir.ActivationFunctionType.Relu,
                                scale=rz[:],
                            )
                        for kt in range(KT):
                            pst = ppool.tile([P, P], F16, name="ps_tr")
                            nc.tensor.transpose(
                                pst[:], x2[:, kt * P : (kt + 1) * P], ident_sb[:]
                            )
                            nc.vector.tensor_copy(
                                x2T_sb[:, kt * SLOTS + b * P : kt * SLOTS + (b + 1) * P],
                                pst[:],
                            )
                    else:
                        of = wpool.tile([P, D], F32, name="of")
                        nc.scalar.activation(
                            of[:],
                            psb[:, 0:D],
                            mybir.ActivationFunctionType.Identity,
                            scale=rz[:],
                        )
                        nc.sync.dma_start(out_dram[b * P : (b + 1) * P, :], of[:])

    nc.compile()
    return nc


def _host_prep(cfg, x, edge_index, params):
    """Build per-core in_maps + unpermute info."""
    N = cfg.N
    src = np.concatenate([edge_index[0], np.arange(N, dtype=edge_index.dtype)])
    dst = np.concatenate([edge_index[1], np.arange(N, dtype=edge_index.dtype)])
    src = src.astype(np.int64)
    dst = dst.astype(np.int64)
    node_loc, node_blk, node_slot, blk_count, C = _partition_graph(cfg, src, dst)
    edge_data = _build_edge_data(
        cfg, src, dst, node_loc, node_blk, node_slot, blk_count
    )

    x_perm_all = np.zeros((cfg.NCORES * cfg.SLOTS, cfg.D), dtype=np.float32)
    x_perm_all[node_loc] = x

    W1, a_s1, a_d1, b1, W2, a_s2, a_d2, b2 = params
    w_tiles = [
        _layout_W(cfg, W1, a_s1, a_d1),
        _layout_W(cfg, W2, a_s2, a_d2),
    ]
    with_bias = bool(np.any(b1) or np.any(b2))
    ident = np.eye(P, dtype=np.float16)

    in_maps = []
    for c in range(cfg.NCORES):
        idx_tile, S, ST = edge_data[c]
        m = {
            "xT": _layout_xT(cfg, x_perm_all[c * cfg.SLOTS : (c + 1) * cfg.SLOTS]),
            "w0": w_tiles[0],
            "w1": w_tiles[1],
            "s": S,
            "st": ST,
            "idx": idx_tile,
            "ident": ident,
        }
        if with_bias:
            m["b0"] = np.tile(b1.astype(np.float32)[None, :], (P, 1))
            m["b1"] = np.tile(b2.astype(np.float32)[None, :], (P, 1))
        in_maps.append(m)
    return in_maps, node_loc, with_bias


_PROGRAM_CACHE = {}


def run_gat(cfg, x, edge_index, params, trace=False):
    in_maps, node_loc, with_bias = _host_prep(cfg, x, edge_index, params)
    key = (cfg.N, cfg.E, cfg.D, cfg.C, with_bias)
    if key not in _PROGRAM_CACHE:
        _PROGRAM_CACHE[key] = _build_program(cfg, with_bias)
    nc = _PROGRAM_CACHE[key]
    res = bass_utils.run_bass_kernel_spmd(
        nc, in_maps, core_ids=list(range(cfg.NCORES)), trace=trace
    )
    outs = np.concatenate([res.results[c]["out"] for c in range(cfg.NCORES)], axis=0)
    full = outs[node_loc]  # unpermute (drops pad slots)
    return full.astype(np.float32), res


def kernel(x, edge_index, W1, att_src1, att_dst1, b1, W2, att_src2, att_dst2, b2):
    x = np.asarray(x, dtype=np.float32)
    edge_index = np.asarray(edge_index)
    cfg = GATConfig(x.shape[0], edge_index.shape[1], x.shape[1])
    params = tuple(
        np.asarray(p, dtype=np.float32)
        for p in (W1, att_src1, att_dst1, b1, W2, att_src2, att_dst2, b2)
    )
    out, _ = run_gat(cfg, x, edge_index, params)
    return out
